# revision 52
# baseline (speedup 1.0000x reference)
"""Trainium2 Bass kernel for location-sensitive attention.

alpha = softmax(w_score . tanh(enc @ W_enc + b_enc + h @ W_dec + conv(prev_alpha) @ W_c2s)) * mask

Sharding: data-parallel over batch B=32 across 8 cores (4 batches/core).
All weights replicated. Full inputs in, full output out.

v2 design (vs v1): all layout work moved to the host so the device does a
pure matmul stream.
  - enc is pre-transposed + pre-cast on host to encT [BPC, KCH, 128, T]
    bf16 -> no PE transposes, no DVE casts, no PSUM->SBUF assembly copies,
    and half the HBM traffic.
  - Output tile layout pre[t<=128, a=512]: lhsT = encT chunk [128k, t],
    rhs = W chunk [128k, 512a]. 8 accumulating matmuls per t-chunk.
  - conv term AND the (dec_e + b_enc) bias are folded into one extra
    matmul per t-chunk: lhsT = Hext_b [104, t] (4 batch-indicator rows +
    100 Hankel rows of padded alpha), rhs = shared Mext [104, 512]
    (4 per-batch bias rows dec_e+b_enc at partitions 0-3 + W_conv.T@W_c2s
    rows at 4-103). The indicator rows select the right bias row, so no
    cross-partition data movement is ever needed. Hankel+indicators built
    on host; Mext on device in one PSUM accumulation group.
  - score e[t] = sum_a w[a]*tanh(pre[t,a]) via DVE tensor_tensor_reduce
    against a host-replicated w_score [128, 512] -- no PE involvement.
  - softmax tail: ACT exp on e [128, 16]; DVE ttr for mask-mult + row sums;
    partition total via a [128x128] ones-matmul that broadcasts the sum to
    all partitions; DVE reciprocal + scale.
  - alpha leaves the device as [128, 16] t-chunk-major tiles; the host
    undoes the layout.
"""

import os
import sys
import numpy as np

for _p in ("/opt/trn_rl_repo", "/root/.axon_site/_ro/trn_rl_repo"):
    if os.path.isdir(_p) and _p not in sys.path:
        sys.path.append(_p)

import ml_dtypes
import concourse.bass as bass
import concourse.bacc as bacc
import concourse.mybir as mybir
from concourse import bass_isa
from concourse import tile

B, T, ENC2, DEC, ATTN = 32, 2000, 1024, 512, 512
NK, KW, PAD = 10, 100, 50
NCORES = 8
BPC = B // NCORES  # batches per core
TP = T + KW  # padded alpha length (50 + 2000 + 50)

F32 = mybir.dt.float32
BF16 = mybir.dt.bfloat16
AF = mybir.ActivationFunctionType
ALU = mybir.AluOpType
BF = ml_dtypes.bfloat16

KCH = ENC2 // 128  # 8 k-chunks of the 1024 contraction
NC_T = 16          # t-chunks of 128 (last is 80)
HR = KW + BPC      # 4 batch-indicator rows + 100 Hankel rows

# enc is stored per batch as three t-thirds, each contiguous in DRAM and
# SBUF: cols = third_base + ki*third_len + (t - third_t0).  Chunk boundaries
# (multiples of 128) never straddle a third.
THIRDS = ((0, 512), (512, 1280), (1280, T))  # (t0, t1)
TH_LEN = tuple(t1 - t0 for t0, t1 in THIRDS)
TH_BASE = (0, KCH * TH_LEN[0], KCH * (TH_LEN[0] + TH_LEN[1]))


def _tt(c):
    return 128 if c < NC_T - 1 else T - 128 * (NC_T - 1)  # 80


def _ecol(c, ki):
    """et column of (t-chunk c, contraction chunk ki) in the thirds layout."""
    th = 0 if c < 4 else (1 if c < 10 else 2)
    return TH_BASE[th] + ki * TH_LEN[th] + (c * 128 - THIRDS[th][0])


def build_nc():
    nc = bacc.Bacc(None, target_bir_lowering=False)

    # blob1 [128, 4*ATTN + 4*HR + ATTN]: wdec | hT | wbc   (dense, 128 part)
    # blob2 [NK, HR + ATTN + ATTN]: wc | wcs | bencr(row0) (small, 10 part)
    B1W = 4 * ATTN + 4 * HR + ATTN
    B2W = HR + 2 * ATTN
    encT = nc.declare_dram_parameter("encT", [BPC, 128, KCH * T], BF16, isOutput=False)
    hext = nc.declare_dram_parameter("hext", [BPC, HR, T], BF16, isOutput=False)
    wsb = nc.declare_dram_parameter("wsb", [128, KCH * ATTN], BF16, isOutput=False)
    blob1 = nc.declare_dram_parameter("blob1", [128, B1W], BF16, isOutput=False)
    blob2 = nc.declare_dram_parameter("blob2", [NK, B2W], BF16, isOutput=False)
    maskt = nc.declare_dram_parameter("maskt", [BPC, 128, NC_T], F32, isOutput=False)
    out = nc.declare_dram_parameter("out", [BPC, 128, NC_T], F32, isOutput=True)

    with tile.TileContext(nc) as tc:
        with (
            tc.tile_pool(name="const", bufs=1) as cpool,
            tc.tile_pool(name="enc", bufs=2) as enc_pool,
            tc.tile_pool(name="hx", bufs=2) as hx_pool,
            tc.tile_pool(name="th", bufs=3) as th_pool,
            tc.tile_pool(name="scr", bufs=2) as scr_pool,
            tc.tile_pool(name="tail", bufs=2) as tail_pool,
            tc.tile_pool(name="pacc", bufs=4, space="PSUM") as pacc_pool,
            tc.tile_pool(name="pset", bufs=1, space="PSUM") as pset_pool,
        ):
            # ---- batch-0 DMAs in consumption order.  enc third-0 slices
            # plus wsb are all chunk-0 needs (~2.8 MB); thirds 1 and 2 land
            # while chunks 0-3 / 4-9 compute. ----
            b2_sb = cpool.tile([NK, B2W], BF16)
            nc.sync.dma_start(b2_sb[:, :], blob2[:, :])
            wc_sb = b2_sb[:, 0:HR]
            wcs_sb = b2_sb[:, HR : HR + ATTN]
            bencr_sb = b2_sb[0:1, HR + ATTN : HR + 2 * ATTN]



            sel1h = cpool.tile([1, HR], BF16)
            nc.gpsimd.memset(sel1h[:, :], 0.0)
            nc.gpsimd.memset(sel1h[:, 0:BPC], 1.0)
            ones128 = cpool.tile([128, 128], F32)
            nc.gpsimd.memset(ones128[:, :], 1.0)

            wsb_sb = cpool.tile([128, KCH * ATTN], BF16)
            b1_sb = cpool.tile([128, B1W], BF16)
            wdec_sb = b1_sb[:, 0 : 4 * ATTN]
            ht_sb = b1_sb[:, 4 * ATTN : 4 * ATTN + 4 * HR]
            wbc_sb = b1_sb[:, 4 * ATTN + 4 * HR : B1W]

            for q in range(4):
                nc.sync.dma_start(
                    wsb_sb[:, q * 2 * ATTN : (q + 1) * 2 * ATTN],
                    wsb[:, q * 2 * ATTN : (q + 1) * 2 * ATTN],
                )
            nc.sync.dma_start(b1_sb[:, :], blob1[:, :])

            et0 = enc_pool.tile([128, KCH * T], BF16, tag="encT")
            hx0 = hx_pool.tile([HR, T], BF16, tag="hext")
            mk0 = tail_pool.tile([128, NC_T], F32, tag="mask")
            nc.sync.dma_start(mk0[:, :], maskt[0, :, :])
            for th in range(3):
                for ki in range(KCH):
                    lo = TH_BASE[th] + ki * TH_LEN[th]
                    hi = lo + TH_LEN[th]
                    eng = nc.scalar if ki % 2 == 0 else nc.gpsimd
                    eng.dma_start(et0[:, lo:hi], encT[0, :, lo:hi])
                if th == 0:
                    nc.gpsimd.dma_start(hx0[:, :], hext[0, :, :])

            def load_batch(b):
                et = enc_pool.tile([128, KCH * T], BF16, tag="encT")
                for ki in range(KCH):
                    lo = ki * (KCH * T) // KCH
                    eng = nc.scalar if ki % 2 == 0 else nc.gpsimd
                    eng.dma_start(
                        et[:, lo : lo + 2000], encT[b, :, lo : lo + 2000]
                    )
                hx = hx_pool.tile([HR, T], BF16, tag="hext")
                nc.sync.dma_start(hx[:, :], hext[b, :, :])
                mk = tail_pool.tile([128, NC_T], F32, tag="mask")
                nc.sync.dma_start(mk[:, :], maskt[b, :, :])
                return et, hx, mk

            # ---- setup matmuls into one PSUM group: Mext [HR, ATTN]
            #   rows 0..3  = dec_e[b] + b_enc  (per-batch bias rows)
            #   rows 4..   = M = wconv.T @ wc2s (wc host-padded with 4 zero
            #                cols so its output lands at rows 4..103)
            e_t0 = tail_pool.tile([128, NC_T], F32, tag="e")
            nc.gpsimd.memset(e_t0[:, :], -50.0)

            mext_ps = pset_pool.tile([HR, ATTN], F32, tag="m")
            nc.tensor.matmul(mext_ps[:, :], wc_sb[:, :], wcs_sb[:, :],
                             start=True, stop=False)
            for dc in range(4):
                nc.tensor.matmul(
                    mext_ps[:, :],
                    ht_sb[:, dc * HR : (dc + 1) * HR],
                    wdec_sb[:, dc * ATTN : (dc + 1) * ATTN],
                    start=False, stop=False,
                )
            nc.tensor.matmul(mext_ps[:, :], sel1h[:, :], bencr_sb[:, :],
                             start=False, stop=True)
            mext = cpool.tile([HR, ATTN], BF16)
            nc.scalar.copy(mext[:, :], mext_ps[:, :])

            # ---- shared per-chunk epilogue: tanh -> w-mult -> score-reduce
            def emit_chunk_post(pacc, c, tt, e_t):
                th = th_pool.tile([128, ATTN], BF16)
                nc.scalar.activation(th[0:tt, :], pacc[0:tt, :], AF.Tanh)
                scr = scr_pool.tile([128, ATTN], BF16)
                nc.vector.tensor_mul(scr[0:tt, :], th[0:tt, :], wbc_sb[0:tt, :])
                nc.vector.reduce_sum(
                    e_t[0:tt, c : c + 1], scr[0:tt, :],
                    axis=mybir.AxisListType.X,
                )

            # ---- main loop ----
            def emit_tail(b, e_t, mk):
                u = tail_pool.tile([128, NC_T], F32, tag="u")
                nc.scalar.activation(u[:, :], e_t[:, :], AF.Exp)
                wu = tail_pool.tile([128, NC_T], F32, tag="wu")
                ws = tail_pool.tile([128, 1], F32, tag="ws")
                nc.vector.tensor_mul(wu[:, :], u[:, :], mk[:, :])
                nc.vector.reduce_sum(ws[:, :], wu[:, :], axis=mybir.AxisListType.X)
                tot = pset_pool.tile([128, 1], F32, tag="tot")
                nc.tensor.matmul(tot[:, :], ones128[:, :], ws[:, :],
                                 start=True, stop=True)
                r = tail_pool.tile([128, 1], F32, tag="r")
                nc.vector.reciprocal(r[:, :], tot[:, :])
                al = tail_pool.tile([128, NC_T], F32, tag="al")
                nc.vector.tensor_scalar_mul(al[:, :], wu[:, :], r[:, 0:1])
                nc.sync.dma_start(out[b, :, :], al[:, :])

            pending_tail = None
            prefetched = {}
            for b in range(BPC):
                if b == 0:
                    et, hx, mk, e_t, c0 = et0, hx0, mk0, e_t0, 0
                else:
                    et, hx, mk = prefetched.pop(b)
                    e_t = tail_pool.tile([128, NC_T], F32, tag="e")
                    nc.gpsimd.memset(e_t[:, :], -50.0)
                    c0 = 0
                for c in range(c0, NC_T):
                    tt = _tt(c)
                    pacc = pacc_pool.tile([128, ATTN], F32)
                    for ki in range(KCH):
                        ec = _ecol(c, ki)
                        nc.tensor.matmul(
                            pacc[0:tt, :],
                            et[:, ec : ec + tt],
                            wsb_sb[:, ki * ATTN : (ki + 1) * ATTN],
                            start=(ki == 0), stop=False,
                        )
                    nc.tensor.matmul(
                        pacc[0:tt, :],
                        hx[:, c * 128 : c * 128 + tt],
                        mext[:, :],
                        start=False, stop=True,
                    )
                    # prior batch's tail once this batch's PE stream is rolling
                    if pending_tail is not None and c == c0 + 2:
                        emit_tail(*pending_tail)
                        pending_tail = None
                    # prefetch next batch only after this batch's data has
                    # fully landed, so it cannot steal HBM bandwidth from it
                    if c == (10 if b == 0 else 6) and b + 1 < BPC:
                        prefetched[b + 1] = load_batch(b + 1)
                    emit_chunk_post(pacc, c, tt, e_t)
                pending_tail = (b, e_t, mk)
            if pending_tail is not None:
                emit_tail(*pending_tail)

    nc.compile()
    return nc


_NC_CACHE = None


def get_nc():
    global _NC_CACHE
    if _NC_CACHE is None:
        _NC_CACHE = build_nc()
    return _NC_CACHE


def make_in_maps(enc_output, prev_dec_hidden, prev_alpha, mask,
                 W_conv, W_c2s, W_enc, b_enc, W_dec, w_score):
    enc_output = np.asarray(enc_output, np.float32)
    h = np.asarray(prev_dec_hidden, np.float32)
    pa = np.asarray(prev_alpha, np.float32)
    mask = np.asarray(mask, np.float32)

    # encT [B, 128, KCH*T] bf16 in the thirds layout: for each third,
    # block[p, ki, t] = enc[b, t0+t, ki*128+p], flattened per partition
    encT = np.empty((B, 128, KCH * T), BF)
    for th, (t0, t1) in enumerate(THIRDS):
        tl = t1 - t0
        blk = enc_output[:, t0:t1, :].reshape(B, tl, KCH, 128)
        encT[:, :, TH_BASE[th] : TH_BASE[th] + KCH * tl] = (
            blk.transpose(0, 3, 2, 1).reshape(B, 128, KCH * tl).astype(BF)
        )

    # hext[b]: rows 0..3 = batch-indicator (ones at row b%BPC), rows 4..103 =
    # Hankel of padded alpha: hext[b, 4+j, t] = apad[b, j + t]
    apad = np.zeros((B, TP), np.float32)
    apad[:, PAD : PAD + T] = pa[:, 0, :]
    hx = np.lib.stride_tricks.sliding_window_view(apad, T, axis=1)  # [B, KW+1, T]
    hext = np.zeros((B, HR, T), BF)
    for b in range(B):
        hext[b, b % BPC, :] = np.float32(1.0)
    hext[:, BPC : BPC + KW, :] = hx[:, 0:KW, :].astype(BF)

    # W_enc packed k-chunk-major: wsb[p, ki*ATTN + a] = W_enc[ki*128 + p, a]
    wsb = np.ascontiguousarray(
        np.asarray(W_enc, np.float32)
        .reshape(KCH, 128, ATTN).transpose(1, 0, 2).reshape(128, KCH * ATTN)
    ).astype(BF)
    wbc = np.ascontiguousarray(
        np.broadcast_to(np.asarray(w_score, np.float32)[None, :], (128, ATTN))
    ).astype(BF)
    wdecp = np.ascontiguousarray(
        np.asarray(W_dec, np.float32)
        .reshape(4, 128, ATTN).transpose(1, 0, 2).reshape(128, 4 * ATTN)
    ).astype(BF)
    # blob2 [NK, HR + 2*ATTN]: wc (padded with BPC zero cols so M lands at
    # Mext rows BPC..) | wcs | bencr at row 0 of the last block
    B2W = HR + 2 * ATTN
    blob2 = np.zeros((NK, B2W), BF)
    blob2[:, BPC:HR] = np.asarray(W_conv, np.float32).reshape(NK, KW).astype(BF)
    blob2[:, HR : HR + ATTN] = np.asarray(W_c2s, np.float32).astype(BF)
    blob2[0, HR + ATTN :] = np.asarray(b_enc, np.float32).astype(BF)
    # mask in t-chunk-major tile layout with zero padding
    maskt = np.zeros((B, 128, NC_T), np.float32)
    mpad = np.zeros((B, NC_T * 128), np.float32)
    mpad[:, :T] = mask
    maskt[:, :, :] = mpad.reshape(B, NC_T, 128).transpose(0, 2, 1)

    in_maps = []
    for cix in range(NCORES):
        s = slice(cix * BPC, (cix + 1) * BPC)
        # blob1 [128, 4*ATTN + 4*HR + ATTN]: wdec | hT | wbc.  hT is packed
        # d-chunk-major, cols c*HR+0..3 hold the hidden state, c*HR+4.. are
        # zero (keeps the Mext accumulation group full-range).
        B1W = 4 * ATTN + 4 * HR + ATTN
        blob1 = np.zeros((128, B1W), BF)
        blob1[:, 0 : 4 * ATTN] = wdecp
        hTc = h[s].astype(BF).T.reshape(4, 128, BPC)
        for c in range(4):
            blob1[:, 4 * ATTN + c * HR : 4 * ATTN + c * HR + BPC] = hTc[c]
        blob1[:, 4 * ATTN + 4 * HR :] = wbc
        in_maps.append(
            {
                "encT": np.ascontiguousarray(encT[s]),
                "hext": np.ascontiguousarray(hext[s]),
                "wsb": wsb,
                "blob1": blob1,
                "blob2": blob2,
                "maskt": np.ascontiguousarray(maskt[s]),
            }
        )
    return in_maps


def assemble_output(results) -> np.ndarray:
    outs = [np.asarray(results[c]["out"], np.float32) for c in range(NCORES)]
    full = np.concatenate(outs, axis=0)  # [B, 128, NC_T]
    alpha = full.transpose(0, 2, 1).reshape(B, NC_T * 128)[:, :T]
    return np.ascontiguousarray(alpha).reshape(B, 1, T)


def kernel(**inputs) -> np.ndarray:
    from concourse.bass_utils import run_bass_kernel_spmd

    nc = get_nc()
    in_maps = make_in_maps(**inputs)
    res = run_bass_kernel_spmd(nc, in_maps, core_ids=list(range(NCORES)))
    return assemble_output(res.results)


# revision 55
# speedup vs baseline: 1.1532x; 1.1532x over previous
"""Trainium2 Bass kernel for location-sensitive attention.

alpha = softmax(w_score . tanh(enc @ W_enc + b_enc + h @ W_dec + conv(prev_alpha) @ W_c2s)) * mask

Sharding: data-parallel over batch B=32 across 8 cores (4 batches/core).
All weights replicated. Full inputs in, full output out.

v2 design (vs v1): all layout work moved to the host so the device does a
pure matmul stream.
  - enc is pre-transposed + pre-cast on host to encT [BPC, KCH, 128, T]
    bf16 -> no PE transposes, no DVE casts, no PSUM->SBUF assembly copies,
    and half the HBM traffic.
  - Output tile layout pre[t<=128, a=512]: lhsT = encT chunk [128k, t],
    rhs = W chunk [128k, 512a]. 8 accumulating matmuls per t-chunk.
  - conv term AND the (dec_e + b_enc) bias are folded into one extra
    matmul per t-chunk: lhsT = Hext_b [104, t] (4 batch-indicator rows +
    100 Hankel rows of padded alpha), rhs = shared Mext [104, 512]
    (4 per-batch bias rows dec_e+b_enc at partitions 0-3 + W_conv.T@W_c2s
    rows at 4-103). The indicator rows select the right bias row, so no
    cross-partition data movement is ever needed. Hankel+indicators built
    on host; Mext on device in one PSUM accumulation group.
  - score e[t] = sum_a w[a]*tanh(pre[t,a]) via DVE tensor_tensor_reduce
    against a host-replicated w_score [128, 512] -- no PE involvement.
  - softmax tail: ACT exp on e [128, 16]; DVE ttr for mask-mult + row sums;
    partition total via a [128x128] ones-matmul that broadcasts the sum to
    all partitions; DVE reciprocal + scale.
  - alpha leaves the device as [128, 16] t-chunk-major tiles; the host
    undoes the layout.
"""

import os
import sys
import numpy as np

for _p in ("/opt/trn_rl_repo", "/root/.axon_site/_ro/trn_rl_repo"):
    if os.path.isdir(_p) and _p not in sys.path:
        sys.path.append(_p)

import ml_dtypes
import concourse.bass as bass
import concourse.bacc as bacc
import concourse.mybir as mybir
from concourse import bass_isa
from concourse import tile

B, T, ENC2, DEC, ATTN = 32, 2000, 1024, 512, 512
NK, KW, PAD = 10, 100, 50
NCORES = 8
BPC = B // NCORES  # batches per core
TP = T + KW  # padded alpha length (50 + 2000 + 50)

F32 = mybir.dt.float32
BF16 = mybir.dt.bfloat16
AF = mybir.ActivationFunctionType
ALU = mybir.AluOpType
BF = ml_dtypes.bfloat16

KCH = ENC2 // 128  # 8 k-chunks of the 1024 contraction
NC_T = 16          # t-chunks of 128 (last is 80)
HR = KW + BPC      # 4 batch-indicator rows + 100 Hankel rows

# enc is stored per batch as three t-thirds, each contiguous in DRAM and
# SBUF: cols = third_base + ki*third_len + (t - third_t0).  Chunk boundaries
# (multiples of 128) never straddle a third.
THIRDS = ((0, 512), (512, 1280), (1280, T))  # (t0, t1)
TH_LEN = tuple(t1 - t0 for t0, t1 in THIRDS)
TH_BASE = (0, KCH * TH_LEN[0], KCH * (TH_LEN[0] + TH_LEN[1]))


def _tt(c):
    return 128 if c < NC_T - 1 else T - 128 * (NC_T - 1)  # 80


def _ecol(c, ki):
    """et column of (t-chunk c, contraction chunk ki) in the thirds layout."""
    th = 0 if c < 4 else (1 if c < 10 else 2)
    return TH_BASE[th] + ki * TH_LEN[th] + (c * 128 - THIRDS[th][0])


def build_nc():
    nc = bacc.Bacc(None, target_bir_lowering=False)

    # blob1 [128, 4*ATTN + 4*HR + ATTN]: wdec | hT | wbc   (dense, 128 part)
    # blob2 [NK, HR + ATTN + ATTN]: wc | wcs | bencr(row0) (small, 10 part)
    B1W = 4 * ATTN + 4 * HR + ATTN
    B2W = HR + 2 * ATTN
    encT = nc.declare_dram_parameter("encT", [BPC, 128, KCH * T], BF16, isOutput=False)
    hext = nc.declare_dram_parameter("hext", [BPC, HR, T], BF16, isOutput=False)
    wsb = nc.declare_dram_parameter("wsb", [128, KCH * ATTN], BF16, isOutput=False)
    blob1 = nc.declare_dram_parameter("blob1", [128, B1W], BF16, isOutput=False)
    blob2 = nc.declare_dram_parameter("blob2", [NK, B2W], BF16, isOutput=False)
    maskt = nc.declare_dram_parameter("maskt", [BPC, 128, NC_T], F32, isOutput=False)
    out = nc.declare_dram_parameter("out", [BPC, 128, NC_T], F32, isOutput=True)

    with tile.TileContext(nc) as tc:
        with (
            tc.tile_pool(name="const", bufs=1) as cpool,
            tc.tile_pool(name="enc", bufs=2) as enc_pool,
            tc.tile_pool(name="hx", bufs=2) as hx_pool,
            tc.tile_pool(name="th", bufs=3) as th_pool,
            tc.tile_pool(name="scr", bufs=2) as scr_pool,
            tc.tile_pool(name="tail", bufs=2) as tail_pool,
            tc.tile_pool(name="pacc", bufs=4, space="PSUM") as pacc_pool,
            tc.tile_pool(name="pset", bufs=1, space="PSUM") as pset_pool,
        ):
            # ---- batch-0 DMAs in consumption order.  enc third-0 slices
            # plus wsb are all chunk-0 needs (~2.8 MB); thirds 1 and 2 land
            # while chunks 0-3 / 4-9 compute. ----
            b2_sb = cpool.tile([NK, B2W], BF16)
            nc.sync.dma_start(b2_sb[:, :], blob2[:, :])
            wc_sb = b2_sb[:, 0:HR]
            wcs_sb = b2_sb[:, HR : HR + ATTN]
            bencr_sb = b2_sb[0:1, HR + ATTN : HR + 2 * ATTN]



            sel1h = cpool.tile([1, HR], BF16)
            nc.gpsimd.memset(sel1h[:, :], 0.0)
            nc.gpsimd.memset(sel1h[:, 0:BPC], 1.0)
            ones128 = cpool.tile([128, 128], F32)
            nc.gpsimd.memset(ones128[:, :], 1.0)

            wsb_sb = cpool.tile([128, KCH * ATTN], BF16)
            b1_sb = cpool.tile([128, B1W], BF16)
            wdec_sb = b1_sb[:, 0 : 4 * ATTN]
            ht_sb = b1_sb[:, 4 * ATTN : 4 * ATTN + 4 * HR]
            wbc_sb = b1_sb[:, 4 * ATTN + 4 * HR : B1W]

            for q in range(4):
                nc.sync.dma_start(
                    wsb_sb[:, q * 2 * ATTN : (q + 1) * 2 * ATTN],
                    wsb[:, q * 2 * ATTN : (q + 1) * 2 * ATTN],
                )
            nc.sync.dma_start(b1_sb[:, :], blob1[:, :])

            et0 = enc_pool.tile([128, KCH * T], BF16, tag="encT")
            hx0 = hx_pool.tile([HR, T], BF16, tag="hext")
            mk0 = tail_pool.tile([128, NC_T], F32, tag="mask")
            nc.sync.dma_start(mk0[:, :], maskt[0, :, :])
            for th in range(3):
                for ki in range(KCH):
                    lo = TH_BASE[th] + ki * TH_LEN[th]
                    hi = lo + TH_LEN[th]
                    eng = nc.scalar if ki % 2 == 0 else nc.gpsimd
                    eng.dma_start(et0[:, lo:hi], encT[0, :, lo:hi])
                if th == 0:
                    nc.gpsimd.dma_start(hx0[:, :], hext[0, :, :])

            def load_batch(b):
                et = enc_pool.tile([128, KCH * T], BF16, tag="encT")
                for ki in range(KCH):
                    lo = ki * (KCH * T) // KCH
                    eng = nc.scalar if ki % 2 == 0 else nc.gpsimd
                    eng.dma_start(
                        et[:, lo : lo + 2000], encT[b, :, lo : lo + 2000]
                    )
                hx = hx_pool.tile([HR, T], BF16, tag="hext")
                nc.sync.dma_start(hx[:, :], hext[b, :, :])
                mk = tail_pool.tile([128, NC_T], F32, tag="mask")
                nc.sync.dma_start(mk[:, :], maskt[b, :, :])
                return et, hx, mk

            # ---- setup matmuls into one PSUM group: Mext [HR, ATTN]
            #   rows 0..3  = dec_e[b] + b_enc  (per-batch bias rows)
            #   rows 4..   = M = wconv.T @ wc2s (wc host-padded with 4 zero
            #                cols so its output lands at rows 4..103)
            e_t0 = tail_pool.tile([128, NC_T], F32, tag="e")
            nc.gpsimd.memset(e_t0[:, :], -50.0)

            mext_ps = pset_pool.tile([HR, ATTN], F32, tag="m")
            nc.tensor.matmul(mext_ps[:, :], wc_sb[:, :], wcs_sb[:, :],
                             start=True, stop=False)
            for dc in range(4):
                nc.tensor.matmul(
                    mext_ps[:, :],
                    ht_sb[:, dc * HR : (dc + 1) * HR],
                    wdec_sb[:, dc * ATTN : (dc + 1) * ATTN],
                    start=False, stop=False,
                )
            nc.tensor.matmul(mext_ps[:, :], sel1h[:, :], bencr_sb[:, :],
                             start=False, stop=True)
            mext = cpool.tile([HR, ATTN], BF16)
            nc.scalar.copy(mext[:, :], mext_ps[:, :])

            # ---- shared per-chunk epilogue: tanh -> w-mult -> score-reduce
            def emit_chunk_post(pacc, c, tt, e_t):
                th = th_pool.tile([128, ATTN], BF16)
                nc.scalar.activation(th[0:tt, :], pacc[0:tt, :], AF.Tanh)
                scr = scr_pool.tile([128, ATTN], BF16)
                nc.vector.tensor_mul(scr[0:tt, :], th[0:tt, :], wbc_sb[0:tt, :])
                nc.vector.reduce_sum(
                    e_t[0:tt, c : c + 1], scr[0:tt, :],
                    axis=mybir.AxisListType.X,
                )

            # ---- main loop ----
            def emit_tail_a(b, e_t, mk):
                u = tail_pool.tile([128, NC_T], F32, tag="u")
                nc.scalar.activation(u[:, :], e_t[:, :], AF.Exp)
                wu = tail_pool.tile([128, NC_T], F32, tag="wu")
                ws = tail_pool.tile([128, 1], F32, tag="ws")
                nc.vector.tensor_mul(wu[:, :], u[:, :], mk[:, :])
                nc.vector.reduce_sum(ws[:, :], wu[:, :], axis=mybir.AxisListType.X)
                return b, wu, ws

            def emit_tail_b(b, wu, ws):
                tot = pset_pool.tile([128, 1], F32, tag="tot")
                nc.tensor.matmul(tot[:, :], ones128[:, :], ws[:, :],
                                 start=True, stop=True)
                r = tail_pool.tile([128, 1], F32, tag="r")
                nc.vector.reciprocal(r[:, :], tot[:, :])
                al = tail_pool.tile([128, NC_T], F32, tag="al")
                nc.vector.tensor_scalar_mul(al[:, :], wu[:, :], r[:, 0:1])
                nc.sync.dma_start(out[b, :, :], al[:, :])

            def emit_tail(b, e_t, mk):
                emit_tail_b(*emit_tail_a(b, e_t, mk))

            pending_tail = None
            tail_mid = None
            prefetched = {}
            for b in range(BPC):
                if b == 0:
                    et, hx, mk, e_t, c0 = et0, hx0, mk0, e_t0, 0
                else:
                    et, hx, mk = prefetched.pop(b)
                    e_t = tail_pool.tile([128, NC_T], F32, tag="e")
                    nc.gpsimd.memset(e_t[:, :], -50.0)
                    c0 = 0
                for c in range(c0, NC_T):
                    tt = _tt(c)
                    pacc = pacc_pool.tile([128, ATTN], F32)
                    for ki in range(KCH):
                        ec = _ecol(c, ki)
                        nc.tensor.matmul(
                            pacc[0:tt, :],
                            et[:, ec : ec + tt],
                            wsb_sb[:, ki * ATTN : (ki + 1) * ATTN],
                            start=(ki == 0), stop=False,
                        )
                    nc.tensor.matmul(
                        pacc[0:tt, :],
                        hx[:, c * 128 : c * 128 + tt],
                        mext[:, :],
                        start=False, stop=True,
                    )
                    # prior batch's tail once this batch's PE stream is
                    # rolling: DVE part early, PE/DMA part later where
                    # nothing can stall on it
                    if pending_tail is not None and c == c0 + 2:
                        tail_mid = emit_tail_a(*pending_tail)
                        pending_tail = None
                    if tail_mid is not None and c == c0 + 6:
                        emit_tail_b(*tail_mid)
                        tail_mid = None
                    # prefetch next batch only after this batch's data has
                    # fully landed, so it cannot steal HBM bandwidth from it
                    if c == (10 if b == 0 else 6) and b + 1 < BPC:
                        prefetched[b + 1] = load_batch(b + 1)
                    emit_chunk_post(pacc, c, tt, e_t)
                pending_tail = (b, e_t, mk)
            if pending_tail is not None:
                emit_tail(*pending_tail)

    nc.compile()
    return nc


_NC_CACHE = None


def get_nc():
    global _NC_CACHE
    if _NC_CACHE is None:
        _NC_CACHE = build_nc()
    return _NC_CACHE


def make_in_maps(enc_output, prev_dec_hidden, prev_alpha, mask,
                 W_conv, W_c2s, W_enc, b_enc, W_dec, w_score):
    enc_output = np.asarray(enc_output, np.float32)
    h = np.asarray(prev_dec_hidden, np.float32)
    pa = np.asarray(prev_alpha, np.float32)
    mask = np.asarray(mask, np.float32)

    # encT [B, 128, KCH*T] bf16 in the thirds layout: for each third,
    # block[p, ki, t] = enc[b, t0+t, ki*128+p], flattened per partition
    encT = np.empty((B, 128, KCH * T), BF)
    for th, (t0, t1) in enumerate(THIRDS):
        tl = t1 - t0
        blk = enc_output[:, t0:t1, :].reshape(B, tl, KCH, 128)
        encT[:, :, TH_BASE[th] : TH_BASE[th] + KCH * tl] = (
            blk.transpose(0, 3, 2, 1).reshape(B, 128, KCH * tl).astype(BF)
        )

    # hext[b]: rows 0..3 = batch-indicator (ones at row b%BPC), rows 4..103 =
    # Hankel of padded alpha: hext[b, 4+j, t] = apad[b, j + t]
    apad = np.zeros((B, TP), np.float32)
    apad[:, PAD : PAD + T] = pa[:, 0, :]
    hx = np.lib.stride_tricks.sliding_window_view(apad, T, axis=1)  # [B, KW+1, T]
    hext = np.zeros((B, HR, T), BF)
    for b in range(B):
        hext[b, b % BPC, :] = np.float32(1.0)
    hext[:, BPC : BPC + KW, :] = hx[:, 0:KW, :].astype(BF)

    # W_enc packed k-chunk-major: wsb[p, ki*ATTN + a] = W_enc[ki*128 + p, a]
    wsb = np.ascontiguousarray(
        np.asarray(W_enc, np.float32)
        .reshape(KCH, 128, ATTN).transpose(1, 0, 2).reshape(128, KCH * ATTN)
    ).astype(BF)
    wbc = np.ascontiguousarray(
        np.broadcast_to(np.asarray(w_score, np.float32)[None, :], (128, ATTN))
    ).astype(BF)
    wdecp = np.ascontiguousarray(
        np.asarray(W_dec, np.float32)
        .reshape(4, 128, ATTN).transpose(1, 0, 2).reshape(128, 4 * ATTN)
    ).astype(BF)
    # blob2 [NK, HR + 2*ATTN]: wc (padded with BPC zero cols so M lands at
    # Mext rows BPC..) | wcs | bencr at row 0 of the last block
    B2W = HR + 2 * ATTN
    blob2 = np.zeros((NK, B2W), BF)
    blob2[:, BPC:HR] = np.asarray(W_conv, np.float32).reshape(NK, KW).astype(BF)
    blob2[:, HR : HR + ATTN] = np.asarray(W_c2s, np.float32).astype(BF)
    blob2[0, HR + ATTN :] = np.asarray(b_enc, np.float32).astype(BF)
    # mask in t-chunk-major tile layout with zero padding
    maskt = np.zeros((B, 128, NC_T), np.float32)
    mpad = np.zeros((B, NC_T * 128), np.float32)
    mpad[:, :T] = mask
    maskt[:, :, :] = mpad.reshape(B, NC_T, 128).transpose(0, 2, 1)

    in_maps = []
    for cix in range(NCORES):
        s = slice(cix * BPC, (cix + 1) * BPC)
        # blob1 [128, 4*ATTN + 4*HR + ATTN]: wdec | hT | wbc.  hT is packed
        # d-chunk-major, cols c*HR+0..3 hold the hidden state, c*HR+4.. are
        # zero (keeps the Mext accumulation group full-range).
        B1W = 4 * ATTN + 4 * HR + ATTN
        blob1 = np.zeros((128, B1W), BF)
        blob1[:, 0 : 4 * ATTN] = wdecp
        hTc = h[s].astype(BF).T.reshape(4, 128, BPC)
        for c in range(4):
            blob1[:, 4 * ATTN + c * HR : 4 * ATTN + c * HR + BPC] = hTc[c]
        blob1[:, 4 * ATTN + 4 * HR :] = wbc
        in_maps.append(
            {
                "encT": np.ascontiguousarray(encT[s]),
                "hext": np.ascontiguousarray(hext[s]),
                "wsb": wsb,
                "blob1": blob1,
                "blob2": blob2,
                "maskt": np.ascontiguousarray(maskt[s]),
            }
        )
    return in_maps


def assemble_output(results) -> np.ndarray:
    outs = [np.asarray(results[c]["out"], np.float32) for c in range(NCORES)]
    full = np.concatenate(outs, axis=0)  # [B, 128, NC_T]
    alpha = full.transpose(0, 2, 1).reshape(B, NC_T * 128)[:, :T]
    return np.ascontiguousarray(alpha).reshape(B, 1, T)


def kernel(**inputs) -> np.ndarray:
    from concourse.bass_utils import run_bass_kernel_spmd

    nc = get_nc()
    in_maps = make_in_maps(**inputs)
    res = run_bass_kernel_spmd(nc, in_maps, core_ids=list(range(NCORES)))
    return assemble_output(res.results)
